# revision 4
# baseline (speedup 1.0000x reference)
"""OODGAT 2-layer GNN kernel for 8 Trainium2 NeuronCores — v5.

Deltas vs v3 baseline (which was gather-descriptor-bound, GpSimd 75% busy):
  - Per-tile indirect DMAs (1664 x ~1.5us serial) replaced by dma_gather
    instructions covering 8 tiles (1024 rows) each; edge slots per block are
    split into low/high-src tile runs so indices fit int16 (<32768).
  - Table rows padded to 256B/512B multiples for dma_gather (descriptor
    drain rate is size-insensitive, so padding is free).
  - One-hot S and S^T matrices are precomputed on host and streamed
    sequentially from DRAM (bandwidth rides under the descriptor-paced
    gathers), removing the per-tile PE transpose + PSUM copy and the
    per-chunk is_equal builds.
  - Slot layout, S/S^T, and idx grids are shared by both layers.
"""
import numpy as np
import ml_dtypes
from dataclasses import dataclass

import concourse.bass as bass
import concourse.bacc as bacc
import concourse.mybir as mybir
import concourse.tile as tile
from concourse.bass_utils import run_bass_kernel_spmd

F32 = mybir.dt.float32
BF16 = mybir.dt.bfloat16
I16 = mybir.dt.int16
BF = ml_dtypes.bfloat16

E1ROW = 256   # T1 row: [sh1(4) | h1(128) | 0pad] bf16 -> 512B
E2ROW = 128   # T2 row: [sh2(4) | h2(32) | 0pad] bf16 -> 256B
GMAX = 7      # tiles per dma_gather (896 rows; margin below 1024-desc ring)


@dataclass
class Cfg:
    N: int = 50000
    IN: int = 256
    H: int = 4
    C1: int = 32
    C2: int = 8
    NC: int = 8
    CH_MAX: int = 48   # max tiles per chunk
    SPLIT: int = 32768

    @property
    def D1(self):
        return self.H * self.C1

    @property
    def D2(self):
        return self.H * self.C2

    @property
    def SH(self):
        return self.N // self.NC

    @property
    def NBLK(self):
        return (self.SH + 127) // 128


def _wrap_idx16(flat):
    n = len(flat)
    core = flat.reshape(n // 16, 16).T.astype(np.int16)
    return np.tile(core, (8, 1))


def host_prep(cfg: Cfg, x, edge_index, W1, att1, b1, W2, att2, b2):
    N, SH, NBLK, NC, SPLIT = cfg.N, cfg.SH, cfg.NBLK, cfg.NC, cfg.SPLIT
    src = np.asarray(edge_index[0], dtype=np.int64)
    dst = np.asarray(edge_index[1], dtype=np.int64)
    # self-loops handled by the per-block identity matmul (no gather)

    core_of = dst // SH
    per_core = []
    kbL = np.zeros(NBLK, np.int64)
    kbH = np.zeros(NBLK, np.int64)
    for c in range(NC):
        m = core_of == c
        s_c, d_c = src[m], dst[m] - c * SH
        lo = s_c < SPLIT
        per_core.append((s_c, d_c, lo))
        for sel, kb in ((lo, kbL), (~lo, kbH)):
            blkcnt = np.bincount(d_c[sel] // 128, minlength=NBLK)
            np.maximum(kb, blkcnt, out=kb)
    kbL = -(-kbL // 128)
    kbH = -(-kbH // 128)

    # chunks of whole blocks; per chunk tiles ordered [L tiles | H tiles]
    chunks = []   # (b0, b1, t0, ntL, ntH)
    b0 = 0
    t0 = 0
    while b0 < NBLK:
        bend = b0 + 1
        while bend < NBLK and (kbL[b0:bend + 1].sum() + kbH[b0:bend + 1].sum()
                               ) <= cfg.CH_MAX:
            bend += 1
        ntL = int(kbL[b0:bend].sum())
        ntH = int(kbH[b0:bend].sum())
        chunks.append((b0, bend, t0, ntL, ntH))
        t0 += ntL + ntH
        b0 = bend
    T = t0

    ident_b = np.eye(128, dtype=np.float32).astype(BF)
    att1_b = np.broadcast_to(np.asarray(att1, np.float32).reshape(
        1, cfg.D1), (128, cfg.D1)).copy()
    att2_b = np.broadcast_to(np.asarray(att2, np.float32).reshape(
        1, cfg.D2), (128, cfg.D2)).copy()
    b1_b = np.broadcast_to(np.asarray(b1, np.float32).reshape(
        1, cfg.D1), (128, cfg.D1)).copy()
    b2_b = np.broadcast_to(np.asarray(b2, np.float32).reshape(
        1, cfg.C2), (128, cfg.C2)).copy()
    W1_b = np.asarray(W1, np.float32).astype(BF)
    W2_b = np.asarray(W2, np.float32).astype(BF)
    x = np.asarray(x, np.float32)

    in_maps = []
    for c in range(NC):
        s_c, d_c, lo_c = per_core[c]
        idxf = np.zeros(T * 128, np.int64)
        Smat = np.zeros((128, T, 128), np.float32)
        for (b0c, b1c, tc0, ntL, ntH) in chunks:
            tL = tc0
            tH = tc0 + ntL
            for b in range(b0c, b1c):
                for lo, kb, tref in ((True, kbL, 'L'), (False, kbH, 'H')):
                    m = (d_c // 128 == b) & (lo_c == lo)
                    sv = s_c[m] - (0 if lo else SPLIT)
                    ld = d_c[m] - b * 128
                    t0b = tL if lo else tH
                    n = len(sv)
                    lanes = np.arange(n) % 128
                    cols = t0b + np.arange(n) // 128
                    idxf[cols * 128 + lanes] = sv
                    Smat[lanes, cols, ld] = 1.0
                    if lo:
                        tL += int(kb[b])
                    else:
                        tH += int(kb[b])
        idx16 = _wrap_idx16(idxf)
        Sb = Smat.astype(BF)
        STb = np.ascontiguousarray(Smat.transpose(2, 1, 0)).astype(BF)
        xT = np.ascontiguousarray(x[c * SH:(c + 1) * SH].T).astype(BF)
        in_maps.append(dict(
            xT=xT, idx16=idx16,
            Sm=Sb.reshape(128, T * 128),
            STm=STb.reshape(128, T * 128),
            ident=ident_b, W1=W1_b, W2=W2_b,
            att1_b=att1_b, att2_b=att2_b, b1_b=b1_b, b2_b=b2_b,
        ))
    layout = (tuple(int(v) for v in kbL), tuple(int(v) for v in kbH),
              tuple(chunks))
    return in_maps, layout


def build(cfg: Cfg, layout):
    kbL, kbH, chunks = layout
    N, SH, NBLK, NC = cfg.N, cfg.SH, cfg.NBLK, cfg.NC
    IN, H, C1, C2, D1, D2 = cfg.IN, cfg.H, cfg.C1, cfg.C2, cfg.D1, cfg.D2
    SPLIT = cfg.SPLIT
    RW1 = H + D1            # 132
    RW2 = H + D2            # 36
    T = sum(kbL) + sum(kbH)
    CHMAX = max(c[3] + c[4] for c in chunks)
    KIN = IN // 128

    nc = bacc.Bacc("TRN2", target_bir_lowering=False, debug=False,
                   enable_asserts=True, num_devices=NC)

    xT_in = nc.dram_tensor("xT", [IN, SH], BF16, kind="ExternalInput")
    idx_in = nc.dram_tensor("idx16", [128, T * 8], I16, kind="ExternalInput")
    S_in = nc.dram_tensor("Sm", [128, T * 128], BF16, kind="ExternalInput")
    ST_in = nc.dram_tensor("STm", [128, T * 128], BF16, kind="ExternalInput")
    ident_in = nc.dram_tensor("ident", [128, 128], BF16, kind="ExternalInput")
    W1_in = nc.dram_tensor("W1", [IN, D1], BF16, kind="ExternalInput")
    W2_in = nc.dram_tensor("W2", [D1, D2], BF16, kind="ExternalInput")
    att1_in = nc.dram_tensor("att1_b", [128, D1], F32, kind="ExternalInput")
    att2_in = nc.dram_tensor("att2_b", [128, D2], F32, kind="ExternalInput")
    b1_in = nc.dram_tensor("b1_b", [128, D1], F32, kind="ExternalInput")
    b2_in = nc.dram_tensor("b2_b", [128, C2], F32, kind="ExternalInput")
    out = nc.dram_tensor("out", [SH, C2], F32, kind="ExternalOutput")

    T1S = nc.dram_tensor("T1S", [SH, E1ROW], BF16, kind="Internal")
    T1F = nc.dram_tensor("T1F", [N, E1ROW], BF16, kind="Internal",
                         addr_space="Shared")
    T2S = nc.dram_tensor("T2S", [SH, E2ROW], BF16, kind="Internal")
    T2F = nc.dram_tensor("T2F", [N, E2ROW], BF16, kind="Internal",
                         addr_space="Shared")

    with tile.TileContext(nc) as tc:
        with tc.tile_pool(name="res", bufs=1) as res, \
             tc.tile_pool(name="gp", bufs=3) as gp, \
             tc.tile_pool(name="sp", bufs=2) as sp, \
             tc.tile_pool(name="stp", bufs=2) as stp, \
             tc.tile_pool(name="wk", bufs=3) as wk, \
             tc.tile_pool(name="sm", bufs=4) as sm, \
             tc.tile_pool(name="ps_acc", bufs=2, space="PSUM") as ps_acc, \
             tc.tile_pool(name="ps_sd", bufs=2, space="PSUM") as ps_sd, \
             tc.tile_pool(name="ps_mm", bufs=2, space="PSUM") as ps_mm, \
             tc.tile_pool(name="ps_tp", bufs=2, space="PSUM") as ps_tp:

            # ---- resident constants
            idx16 = res.tile([128, T * 8], I16)
            identb = res.tile([128, 128], BF16)
            att1b = res.tile([128, D1], F32)
            att2b = res.tile([128, D2], F32)
            b1b = res.tile([128, D1], F32)
            b2b = res.tile([128, C2], F32)
            W2sb = res.tile([D1, D2], BF16)
            nc.sync.dma_start(idx16[:], idx_in[:])
            nc.sync.dma_start(identb[:], ident_in[:])
            nc.sync.dma_start(att1b[:], att1_in[:])
            nc.sync.dma_start(att2b[:], att2_in[:])
            nc.sync.dma_start(b1b[:], b1_in[:])
            nc.sync.dma_start(b2b[:], b2_in[:])
            nc.sync.dma_start(W2sb[:], W2_in[:])
            xTs, W1s = [], []
            for k in range(KIN):
                t_ = res.tile([128, SH], BF16, tag=f"xT{k}")
                nc.sync.dma_start(t_[:], xT_in[k * 128:(k + 1) * 128, :])
                xTs.append(t_)
                w_ = res.tile([128, D1], BF16, tag=f"W1{k}")
                nc.sync.dma_start(w_[:], W1_in[k * 128:(k + 1) * 128, :])
                W1s.append(w_)

            # ---- phase 1: rows [tanh(.5*h1.att1) | h1 | 0] in bf16
            for i in range(NBLK):
                n0 = i * 128
                P = min(128, SH - n0)
                h1ps = ps_mm.tile([128, D1], F32, space="PSUM", tag="mm")
                for k in range(KIN):
                    nc.tensor.matmul(out=h1ps[:P, :], lhsT=xTs[k][:, n0:n0 + P],
                                     rhs=W1s[k][:], start=(k == 0),
                                     stop=(k == KIN - 1))
                tmp = wk.tile([128, D1], F32, tag="tmp")
                nc.vector.tensor_tensor(out=tmp[:P, :], in0=h1ps[:P, :],
                                        in1=att1b[:P, :],
                                        op=mybir.AluOpType.mult)
                s1 = sm.tile([128, H], F32, tag="s1")
                nc.vector.tensor_reduce(
                    out=s1[:P, :],
                    in_=tmp[:P, :].rearrange("p (h c) -> p h c", h=H),
                    axis=mybir.AxisListType.X, op=mybir.AluOpType.add)
                row = wk.tile([128, E1ROW], BF16, tag="row")
                nc.vector.memset(row[:, RW1:], 0.0)
                nc.scalar.activation(out=row[:P, 0:H], in_=s1[:P, :],
                                     func=mybir.ActivationFunctionType.Tanh,
                                     scale=0.5)
                nc.vector.tensor_copy(out=row[:P, H:RW1], in_=h1ps[:P, :])
                nc.sync.dma_start(T1S[n0:n0 + P, :], row[:P, :])

            def edge_layer(full_tbl, shard_tbl, EROW, D, C, RW, layer):
                TL = full_tbl[0:SPLIT, :]
                TH = full_tbl[SPLIT:N, :]
                for (b0c, b1c, tc0, ntL, ntH) in chunks:
                    nt = ntL + ntH
                    G = gp.tile([128, CHMAX, EROW], BF16, tag="G")
                    for (TB, j0, j1) in ((TL, 0, ntL), (TH, ntL, nt)):
                        j = j0
                        while j < j1:
                            je = min(j + GMAX, j1)
                            nidx = (je - j) * 128
                            nc.gpsimd.dma_gather(
                                G[:, j:je, :], TB[:],
                                idx16[:, (tc0 + j) * 8:(tc0 + je) * 8],
                                nidx, nidx, EROW)
                            j = je
                    S = sp.tile([128, CHMAX, 128], BF16, tag="S")
                    nc.sync.dma_start(
                        S[:, 0:nt, :],
                        S_in[:, tc0 * 128:(tc0 + nt) * 128].rearrange(
                            "p (t d) -> p t d", d=128))
                    ST = stp.tile([128, CHMAX, 128], BF16, tag="ST")
                    nc.sync.dma_start(
                        ST[:, 0:nt, :],
                        ST_in[:, tc0 * 128:(tc0 + nt) * 128].rearrange(
                            "p (t d) -> p t d", d=128))

                    tL = 0
                    tH = ntL
                    for b in range(b0c, b1c):
                        nbL, nbH = kbL[b], kbH[b]
                        tiles = list(range(tL, tL + nbL)) + \
                            list(range(tH, tH + nbH))
                        tL += nbL
                        tH += nbH
                        nb = len(tiles)
                        nb0 = b * 128
                        P = min(128, SH - nb0)
                        sblk = sm.tile([128, H], BF16, tag="sblk")
                        nc.vector.memset(sblk[:], 0.0)
                        nc.scalar.dma_start(sblk[:P, :],
                                            shard_tbl[nb0:nb0 + P, 0:H])
                        Gs = wk.tile([128, RW1], BF16, tag="Gs")
                        nc.vector.memset(Gs[:], 0.0)
                        nc.scalar.dma_start(Gs[:P, 0:RW],
                                            shard_tbl[nb0:nb0 + P, 0:RW])
                        # self-loop: w = exp(0.5*sh^2), rhss = [w | msg*w]
                        ps_self = sm.tile([128, H], BF16, tag="ps_self")
                        nc.vector.tensor_tensor(out=ps_self[:], in0=sblk[:],
                                                in1=sblk[:],
                                                op=mybir.AluOpType.mult)
                        rhss = wk.tile([128, RW1], BF16, tag="rhss")
                        nc.scalar.activation(
                            out=rhss[:, 0:H], in_=ps_self[:],
                            func=mybir.ActivationFunctionType.Exp, scale=0.5)
                        nc.vector.tensor_tensor(
                            out=rhss[:, H:RW].rearrange(
                                "p (h c) -> p h c", h=H),
                            in0=Gs[:, H:RW].rearrange("p (h c) -> p h c", h=H),
                            in1=rhss[:, 0:H, None].to_broadcast([128, H, C]),
                            op=mybir.AluOpType.mult)
                        # dst-side sh expansion via streamed S^T
                        sdps = ps_sd.tile([128, 64 * H], F32, space="PSUM",
                                          tag="sd")
                        for jj, t in enumerate(tiles):
                            nc.tensor.matmul(out=sdps[:, jj * H:(jj + 1) * H],
                                             lhsT=ST[:, t, :], rhs=sblk[:],
                                             start=True, stop=True)
                        # w = exp(0.5*shs*shd); msg *= w  (in place, per tile)
                        for jj, t in enumerate(tiles):
                            nc.vector.tensor_tensor(
                                out=G[:, t, 0:H], in0=G[:, t, 0:H],
                                in1=sdps[:, jj * H:(jj + 1) * H],
                                op=mybir.AluOpType.mult)
                        for jj, t in enumerate(tiles):
                            nc.scalar.activation(
                                out=G[:, t, 0:H], in_=G[:, t, 0:H],
                                func=mybir.ActivationFunctionType.Exp,
                                scale=0.5)
                            nc.vector.tensor_tensor(
                                out=G[:, t, H:RW].rearrange(
                                    "p (h c) -> p h c", h=H),
                                in0=G[:, t, H:RW].rearrange(
                                    "p (h c) -> p h c", h=H),
                                in1=G[:, t, 0:H, None].to_broadcast(
                                    [128, H, C]),
                                op=mybir.AluOpType.mult)
                        acc = ps_acc.tile([128, RW1], F32, space="PSUM",
                                          tag="acc")
                        nc.tensor.matmul(out=acc[:, 0:RW], lhsT=identb[:],
                                         rhs=rhss[:, 0:RW], start=True,
                                         stop=(nb == 0))
                        for jj, t in enumerate(tiles):
                            nc.tensor.matmul(
                                out=acc[:, 0:RW], lhsT=S[:, t, :],
                                rhs=G[:, t, 0:RW], start=False,
                                stop=(jj == nb - 1))
                        if layer == 1:
                            epilogue1(b, acc)
                        else:
                            epilogue2(b, acc)

            def epilogue1(b, acc):
                nb0 = b * 128
                P = min(128, SH - nb0)
                rz = sm.tile([128, H], F32, tag="rz")
                nc.vector.reciprocal(out=rz[:], in_=acc[:, 0:H])
                o1 = wk.tile([128, D1], F32, tag="o1")
                nc.vector.tensor_tensor(
                    out=o1[:].rearrange("p (h c) -> p h c", h=H),
                    in0=acc[:, H:H + D1].rearrange("p (h c) -> p h c", h=H),
                    in1=rz[:, :, None].to_broadcast([128, H, C1]),
                    op=mybir.AluOpType.mult)
                nc.vector.tensor_tensor(out=o1[:], in0=o1[:], in1=b1b[:],
                                        op=mybir.AluOpType.add)
                r1 = wk.tile([128, D1], F32, tag="r1")
                nc.scalar.activation(out=r1[:], in_=o1[:],
                                     func=mybir.ActivationFunctionType.Relu,
                                     scale=-1.0)
                ew = wk.tile([128, D1], F32, tag="ew")
                nc.scalar.activation(out=ew[:], in_=r1[:],
                                     func=mybir.ActivationFunctionType.Exp,
                                     scale=-1.0)
                rp = wk.tile([128, D1], F32, tag="rp")
                nc.scalar.activation(out=rp[:], in_=o1[:],
                                     func=mybir.ActivationFunctionType.Relu)
                hact = wk.tile([128, D1], BF16, tag="hact")
                nc.vector.scalar_tensor_tensor(
                    out=hact[:], in0=ew[:], scalar=-1.0, in1=rp[:],
                    op0=mybir.AluOpType.add, op1=mybir.AluOpType.add)
                tp = ps_tp.tile([128, 128], BF16, space="PSUM", tag="tp")
                nc.tensor.transpose(out=tp[:], in_=hact[:], identity=identb[:])
                hT = wk.tile([128, 128], BF16, tag="hTT")
                nc.vector.tensor_copy(out=hT[:], in_=tp[:])
                h2ps = ps_mm.tile([128, D2], F32, space="PSUM", tag="mm")
                nc.tensor.matmul(out=h2ps[:], lhsT=hT[:], rhs=W2sb[:],
                                 start=True, stop=True)
                t2 = sm.tile([128, D2], F32, tag="t2")
                nc.vector.tensor_tensor(out=t2[:], in0=h2ps[:], in1=att2b[:],
                                        op=mybir.AluOpType.mult)
                s2 = sm.tile([128, H], F32, tag="s2")
                nc.vector.tensor_reduce(
                    out=s2[:], in_=t2[:].rearrange("p (h c) -> p h c", h=H),
                    axis=mybir.AxisListType.X, op=mybir.AluOpType.add)
                row2 = wk.tile([128, E2ROW], BF16, tag="row2")
                nc.vector.memset(row2[:, RW2:], 0.0)
                nc.scalar.activation(out=row2[:, 0:H], in_=s2[:],
                                     func=mybir.ActivationFunctionType.Tanh,
                                     scale=0.5)
                nc.vector.tensor_copy(out=row2[:, H:RW2], in_=h2ps[:])
                nc.sync.dma_start(T2S[nb0:nb0 + P, :], row2[:P, :])

            def epilogue2(b, acc):
                nb0 = b * 128
                P = min(128, SH - nb0)
                rz = sm.tile([128, H], F32, tag="rz")
                nc.vector.reciprocal(out=rz[:], in_=acc[:, 0:H])
                o2 = sm.tile([128, D2], F32, tag="o2")
                nc.vector.tensor_tensor(
                    out=o2[:].rearrange("p (h c) -> p h c", h=H),
                    in0=acc[:, H:H + D2].rearrange("p (h c) -> p h c", h=H),
                    in1=rz[:, :, None].to_broadcast([128, H, C2]),
                    op=mybir.AluOpType.mult)
                red = sm.tile([128, C2], F32, tag="red")
                nc.vector.tensor_reduce(
                    out=red[:], in_=o2[:].rearrange("p (h c) -> p c h", h=H),
                    axis=mybir.AxisListType.X, op=mybir.AluOpType.add)
                fin = sm.tile([128, C2], F32, tag="fin")
                nc.vector.scalar_tensor_tensor(
                    out=fin[:], in0=red[:], scalar=1.0 / H, in1=b2b[:],
                    op0=mybir.AluOpType.mult, op1=mybir.AluOpType.add)
                nc.sync.dma_start(out[nb0:nb0 + P, :], fin[:P, :])

            # ---- layer 1
            nc.gpsimd.collective_compute(
                "AllGather", mybir.AluOpType.bypass,
                replica_groups=[list(range(NC))],
                ins=[T1S[:]], outs=[T1F[:]])
            edge_layer(T1F, T1S, E1ROW, D1, C1, RW1, 1)

            # ---- layer 2
            nc.gpsimd.collective_compute(
                "AllGather", mybir.AluOpType.bypass,
                replica_groups=[list(range(NC))],
                ins=[T2S[:]], outs=[T2F[:]])
            edge_layer(T2F, T2S, E2ROW, D2, C2, RW2, 2)

    nc.compile()
    return nc


_CACHE = {}


def kernel(x, edge_index, W1, att1, b1, W2, att2, b2, cfg: Cfg | None = None,
           trace: bool = False):
    cfg = cfg or Cfg()
    in_maps, layout = host_prep(cfg, x, edge_index, W1, att1, b1, W2, att2, b2)
    key = (cfg.N, cfg.IN, cfg.H, cfg.C1, cfg.C2, layout[0], layout[1])
    if key not in _CACHE:
        _CACHE[key] = build(cfg, layout)
    nc = _CACHE[key]
    r = run_bass_kernel_spmd(nc, in_maps, core_ids=list(range(cfg.NC)),
                             trace=trace)
    outp = np.concatenate([r.results[c]["out"] for c in range(cfg.NC)], axis=0)
    if trace:
        kernel.last_exec_time_ns = r.exec_time_ns
    return outp.astype(np.float32)


# revision 6
# speedup vs baseline: 1.0501x; 1.0501x over previous
"""OODGAT 2-layer GNN kernel for 8 Trainium2 NeuronCores — v5.

Deltas vs v3 baseline (which was gather-descriptor-bound, GpSimd 75% busy):
  - Per-tile indirect DMAs (1664 x ~1.5us serial) replaced by dma_gather
    instructions covering 8 tiles (1024 rows) each; edge slots per block are
    split into low/high-src tile runs so indices fit int16 (<32768).
  - Table rows padded to 256B/512B multiples for dma_gather (descriptor
    drain rate is size-insensitive, so padding is free).
  - One-hot S and S^T matrices are precomputed on host and streamed
    sequentially from DRAM (bandwidth rides under the descriptor-paced
    gathers), removing the per-tile PE transpose + PSUM copy and the
    per-chunk is_equal builds.
  - Slot layout, S/S^T, and idx grids are shared by both layers.
"""
import numpy as np
import ml_dtypes
from dataclasses import dataclass

import concourse.bass as bass
import concourse.bacc as bacc
import concourse.mybir as mybir
import concourse.tile as tile
from concourse.bass_utils import run_bass_kernel_spmd

F32 = mybir.dt.float32
BF16 = mybir.dt.bfloat16
I16 = mybir.dt.int16
BF = ml_dtypes.bfloat16

E1ROW = 256   # T1 row: [sh1(4) | h1(128) | 0pad] bf16 -> 512B
E2ROW = 128   # T2 row: [sh2(4) | h2(32) | 0pad] bf16 -> 256B
GMAX = 7      # tiles per dma_gather (896 rows; margin below 1024-desc ring)
AGB = (0, 13, 26, 39)   # AllGather slice boundaries (blocks)


@dataclass
class Cfg:
    N: int = 50000
    IN: int = 256
    H: int = 4
    C1: int = 32
    C2: int = 8
    NC: int = 8
    CH_MAX: int = 48   # max tiles per chunk
    SPLIT: int = 32768

    @property
    def D1(self):
        return self.H * self.C1

    @property
    def D2(self):
        return self.H * self.C2

    @property
    def SH(self):
        return self.N // self.NC

    @property
    def NBLK(self):
        return (self.SH + 127) // 128


def _wrap_idx16(flat):
    n = len(flat)
    core = flat.reshape(n // 16, 16).T.astype(np.int16)
    return np.tile(core, (8, 1))


def host_prep(cfg: Cfg, x, edge_index, W1, att1, b1, W2, att2, b2):
    N, SH, NBLK, NC, SPLIT = cfg.N, cfg.SH, cfg.NBLK, cfg.NC, cfg.SPLIT
    src = np.asarray(edge_index[0], dtype=np.int64)
    dst = np.asarray(edge_index[1], dtype=np.int64)
    # self-loops handled by the per-block identity matmul (no gather)

    # slice-major permuted row ids so chunked AllGathers write contiguously
    agb = list(AGB) + [NBLK]
    r0s = np.array([a * 128 for a in agb[:-1]])
    r1s = np.array([min(a * 128, SH) for a in agb[1:]])
    srk = r1s - r0s
    offs = np.concatenate([[0], np.cumsum(NC * srk)[:-1]])
    slice_of = np.zeros(SH, np.int64)
    for k in range(len(srk)):
        slice_of[r0s[k]:r1s[k]] = k
    rloc = src % SH
    kk = slice_of[rloc]
    prow = offs[kk] + (src // SH) * srk[kk] + (rloc - r0s[kk])

    core_of = dst // SH
    per_core = []
    kbL = np.zeros(NBLK, np.int64)
    kbH = np.zeros(NBLK, np.int64)
    for c in range(NC):
        m = core_of == c
        s_c, d_c = prow[m], dst[m] - c * SH
        lo = s_c < SPLIT
        per_core.append((s_c, d_c, lo))
        for sel, kb in ((lo, kbL), (~lo, kbH)):
            blkcnt = np.bincount(d_c[sel] // 128, minlength=NBLK)
            np.maximum(kb, blkcnt, out=kb)
    kbL = -(-kbL // 128)
    kbH = -(-kbH // 128)

    # chunks of whole blocks; per chunk tiles ordered [L tiles | H tiles]
    chunks = []   # (b0, b1, t0, ntL, ntH)
    b0 = 0
    t0 = 0
    while b0 < NBLK:
        bend = b0 + 1
        while bend < NBLK and (kbL[b0:bend + 1].sum() + kbH[b0:bend + 1].sum()
                               ) <= cfg.CH_MAX:
            bend += 1
        ntL = int(kbL[b0:bend].sum())
        ntH = int(kbH[b0:bend].sum())
        chunks.append((b0, bend, t0, ntL, ntH))
        t0 += ntL + ntH
        b0 = bend
    T = t0

    ident_b = np.eye(128, dtype=np.float32).astype(BF)
    att1_b = np.broadcast_to(np.asarray(att1, np.float32).reshape(
        1, cfg.D1), (128, cfg.D1)).copy()
    att2_b = np.broadcast_to(np.asarray(att2, np.float32).reshape(
        1, cfg.D2), (128, cfg.D2)).copy()
    b1_b = np.broadcast_to(np.asarray(b1, np.float32).reshape(
        1, cfg.D1), (128, cfg.D1)).copy()
    b2_b = np.broadcast_to(np.asarray(b2, np.float32).reshape(
        1, cfg.C2), (128, cfg.C2)).copy()
    W1_b = np.asarray(W1, np.float32).astype(BF)
    W2_b = np.asarray(W2, np.float32).astype(BF)
    x = np.asarray(x, np.float32)

    in_maps = []
    for c in range(NC):
        s_c, d_c, lo_c = per_core[c]
        idxf = np.zeros(T * 128, np.int64)
        Smat = np.zeros((128, T, 128), np.float32)
        for (b0c, b1c, tc0, ntL, ntH) in chunks:
            tL = tc0
            tH = tc0 + ntL
            for b in range(b0c, b1c):
                for lo, kb, tref in ((True, kbL, 'L'), (False, kbH, 'H')):
                    m = (d_c // 128 == b) & (lo_c == lo)
                    sv = s_c[m] - (0 if lo else SPLIT)
                    ld = d_c[m] - b * 128
                    t0b = tL if lo else tH
                    n = len(sv)
                    lanes = np.arange(n) % 128
                    cols = t0b + np.arange(n) // 128
                    idxf[cols * 128 + lanes] = sv
                    Smat[lanes, cols, ld] = 1.0
                    if lo:
                        tL += int(kb[b])
                    else:
                        tH += int(kb[b])
        idx16 = _wrap_idx16(idxf)
        Sb = Smat.astype(BF)
        STb = np.ascontiguousarray(Smat.transpose(2, 1, 0)).astype(BF)
        xT = np.ascontiguousarray(x[c * SH:(c + 1) * SH].T).astype(BF)
        in_maps.append(dict(
            xT=xT, idx16=idx16,
            Sm=Sb.reshape(128, T * 128),
            STm=STb.reshape(128, T * 128),
            ident=ident_b, W1=W1_b, W2=W2_b,
            att1_b=att1_b, att2_b=att2_b, b1_b=b1_b, b2_b=b2_b,
        ))
    layout = (tuple(int(v) for v in kbL), tuple(int(v) for v in kbH),
              tuple(chunks))
    return in_maps, layout


def build(cfg: Cfg, layout):
    kbL, kbH, chunks = layout
    N, SH, NBLK, NC = cfg.N, cfg.SH, cfg.NBLK, cfg.NC
    IN, H, C1, C2, D1, D2 = cfg.IN, cfg.H, cfg.C1, cfg.C2, cfg.D1, cfg.D2
    SPLIT = cfg.SPLIT
    RW1 = H + D1            # 132
    RW2 = H + D2            # 36
    T = sum(kbL) + sum(kbH)
    CHMAX = max(c[3] + c[4] for c in chunks)
    KIN = IN // 128

    nc = bacc.Bacc("TRN2", target_bir_lowering=False, debug=False,
                   enable_asserts=True, num_devices=NC)

    xT_in = nc.dram_tensor("xT", [IN, SH], BF16, kind="ExternalInput")
    idx_in = nc.dram_tensor("idx16", [128, T * 8], I16, kind="ExternalInput")
    S_in = nc.dram_tensor("Sm", [128, T * 128], BF16, kind="ExternalInput")
    ST_in = nc.dram_tensor("STm", [128, T * 128], BF16, kind="ExternalInput")
    ident_in = nc.dram_tensor("ident", [128, 128], BF16, kind="ExternalInput")
    W1_in = nc.dram_tensor("W1", [IN, D1], BF16, kind="ExternalInput")
    W2_in = nc.dram_tensor("W2", [D1, D2], BF16, kind="ExternalInput")
    att1_in = nc.dram_tensor("att1_b", [128, D1], F32, kind="ExternalInput")
    att2_in = nc.dram_tensor("att2_b", [128, D2], F32, kind="ExternalInput")
    b1_in = nc.dram_tensor("b1_b", [128, D1], F32, kind="ExternalInput")
    b2_in = nc.dram_tensor("b2_b", [128, C2], F32, kind="ExternalInput")
    out = nc.dram_tensor("out", [SH, C2], F32, kind="ExternalOutput")

    T1S = nc.dram_tensor("T1S", [SH, E1ROW], BF16, kind="Internal")
    T1F = nc.dram_tensor("T1F", [N, E1ROW], BF16, kind="Internal",
                         addr_space="Shared")
    T2S = nc.dram_tensor("T2S", [SH, E2ROW], BF16, kind="Internal")
    T2F = nc.dram_tensor("T2F", [N, E2ROW], BF16, kind="Internal",
                         addr_space="Shared")

    with tile.TileContext(nc) as tc:
        with tc.tile_pool(name="res", bufs=1) as res, \
             tc.tile_pool(name="gp", bufs=2) as gp, \
             tc.tile_pool(name="sp", bufs=2) as sp, \
             tc.tile_pool(name="stp", bufs=2) as stp, \
             tc.tile_pool(name="wk", bufs=3) as wk, \
             tc.tile_pool(name="sm", bufs=4) as sm, \
             tc.tile_pool(name="ps_acc", bufs=2, space="PSUM") as ps_acc, \
             tc.tile_pool(name="ps_sd", bufs=2, space="PSUM") as ps_sd, \
             tc.tile_pool(name="ps_mm", bufs=2, space="PSUM") as ps_mm, \
             tc.tile_pool(name="ps_tp", bufs=2, space="PSUM") as ps_tp:

            # ---- resident constants
            idx16 = res.tile([128, T * 8], I16)
            identb = res.tile([128, 128], BF16)
            att1b = res.tile([128, D1], F32)
            att2b = res.tile([128, D2], F32)
            b1b = res.tile([128, D1], F32)
            b2b = res.tile([128, C2], F32)
            W2sb = res.tile([D1, D2], BF16)
            nc.sync.dma_start(idx16[:], idx_in[:])
            nc.sync.dma_start(identb[:], ident_in[:])
            nc.sync.dma_start(att1b[:], att1_in[:])
            nc.sync.dma_start(att2b[:], att2_in[:])
            nc.sync.dma_start(b1b[:], b1_in[:])
            nc.sync.dma_start(b2b[:], b2_in[:])
            nc.sync.dma_start(W2sb[:], W2_in[:])
            xTs, W1s = [], []
            for k in range(KIN):
                t_ = res.tile([128, SH], BF16, tag=f"xT{k}")
                nc.sync.dma_start(t_[:], xT_in[k * 128:(k + 1) * 128, :])
                xTs.append(t_)
                w_ = res.tile([128, D1], BF16, tag=f"W1{k}")
                nc.sync.dma_start(w_[:], W1_in[k * 128:(k + 1) * 128, :])
                W1s.append(w_)

            # AllGather slice boundaries (blocks) for overlap
            agb = list(AGB) + [NBLK]
            ag_r0 = [a * 128 for a in agb[:-1]]
            ag_r1 = [min(a * 128, SH) for a in agb[1:]]
            ag_off = [0]
            for k in range(3):
                ag_off.append(ag_off[-1] + NC * (ag_r1[k] - ag_r0[k]))

            def ag_slice(TS, TF, k):
                r0, r1 = ag_r0[k], ag_r1[k]
                nc.gpsimd.collective_compute(
                    "AllGather", mybir.AluOpType.bypass,
                    replica_groups=[list(range(NC))],
                    ins=[TS[r0:r1, :]],
                    outs=[TF[ag_off[k]:ag_off[k] + NC * (r1 - r0), :]])

            # ---- phase 1: rows [tanh(.5*h1.att1) | h1 | 0] in bf16
            for i in range(NBLK):
                n0 = i * 128
                P = min(128, SH - n0)
                h1ps = ps_mm.tile([128, D1], F32, space="PSUM", tag="mm")
                for k in range(KIN):
                    nc.tensor.matmul(out=h1ps[:P, :], lhsT=xTs[k][:, n0:n0 + P],
                                     rhs=W1s[k][:], start=(k == 0),
                                     stop=(k == KIN - 1))
                tmp = wk.tile([128, D1], F32, tag="tmp")
                nc.vector.tensor_tensor(out=tmp[:P, :], in0=h1ps[:P, :],
                                        in1=att1b[:P, :],
                                        op=mybir.AluOpType.mult)
                s1 = sm.tile([128, H], F32, tag="s1")
                nc.vector.tensor_reduce(
                    out=s1[:P, :],
                    in_=tmp[:P, :].rearrange("p (h c) -> p h c", h=H),
                    axis=mybir.AxisListType.X, op=mybir.AluOpType.add)
                row = wk.tile([128, E1ROW], BF16, tag="row")
                nc.vector.memset(row[:, RW1:], 0.0)
                nc.scalar.activation(out=row[:P, 0:H], in_=s1[:P, :],
                                     func=mybir.ActivationFunctionType.Tanh,
                                     scale=0.5)
                nc.vector.tensor_copy(out=row[:P, H:RW1], in_=h1ps[:P, :])
                nc.sync.dma_start(T1S[n0:n0 + P, :], row[:P, :])
                for k in range(4):
                    if i == agb[k + 1] - 1:
                        ag_slice(T1S, T1F, k)

            def edge_layer(full_tbl, shard_tbl, EROW, D, C, RW, layer):
                TL = full_tbl[0:SPLIT, :]
                TH = full_tbl[SPLIT:N, :]
                for (b0c, b1c, tc0, ntL, ntH) in chunks:
                    nt = ntL + ntH
                    G = gp.tile([128, CHMAX, EROW], BF16, tag="G")
                    for (TB, j0, j1) in ((TL, 0, ntL), (TH, ntL, nt)):
                        j = j0
                        while j < j1:
                            je = min(j + GMAX, j1)
                            nidx = (je - j) * 128
                            nc.gpsimd.dma_gather(
                                G[:, j:je, :], TB[:],
                                idx16[:, (tc0 + j) * 8:(tc0 + je) * 8],
                                nidx, nidx, EROW)
                            j = je
                    S = sp.tile([128, CHMAX, 128], BF16, tag="S")
                    nc.sync.dma_start(
                        S[:, 0:nt, :],
                        S_in[:, tc0 * 128:(tc0 + nt) * 128].rearrange(
                            "p (t d) -> p t d", d=128))
                    ST = stp.tile([128, CHMAX, 128], BF16, tag="ST")
                    nc.sync.dma_start(
                        ST[:, 0:nt, :],
                        ST_in[:, tc0 * 128:(tc0 + nt) * 128].rearrange(
                            "p (t d) -> p t d", d=128))

                    tL = 0
                    tH = ntL
                    for b in range(b0c, b1c):
                        nbL, nbH = kbL[b], kbH[b]
                        tiles = list(range(tL, tL + nbL)) + \
                            list(range(tH, tH + nbH))
                        tL += nbL
                        tH += nbH
                        nb = len(tiles)
                        nb0 = b * 128
                        P = min(128, SH - nb0)
                        sblk = sm.tile([128, H], BF16, tag="sblk")
                        nc.vector.memset(sblk[:], 0.0)
                        nc.scalar.dma_start(sblk[:P, :],
                                            shard_tbl[nb0:nb0 + P, 0:H])
                        Gs = wk.tile([128, RW1], BF16, tag="Gs")
                        nc.vector.memset(Gs[:], 0.0)
                        nc.scalar.dma_start(Gs[:P, 0:RW],
                                            shard_tbl[nb0:nb0 + P, 0:RW])
                        # self-loop: w = exp(0.5*sh^2), rhss = [w | msg*w]
                        ps_self = sm.tile([128, H], BF16, tag="ps_self")
                        nc.vector.tensor_tensor(out=ps_self[:], in0=sblk[:],
                                                in1=sblk[:],
                                                op=mybir.AluOpType.mult)
                        rhss = wk.tile([128, RW1], BF16, tag="rhss")
                        nc.scalar.activation(
                            out=rhss[:, 0:H], in_=ps_self[:],
                            func=mybir.ActivationFunctionType.Exp, scale=0.5)
                        nc.vector.tensor_tensor(
                            out=rhss[:, H:RW].rearrange(
                                "p (h c) -> p h c", h=H),
                            in0=Gs[:, H:RW].rearrange("p (h c) -> p h c", h=H),
                            in1=rhss[:, 0:H, None].to_broadcast([128, H, C]),
                            op=mybir.AluOpType.mult)
                        # dst-side sh expansion via streamed S^T
                        sdps = ps_sd.tile([128, 64 * H], F32, space="PSUM",
                                          tag="sd")
                        for jj, t in enumerate(tiles):
                            nc.tensor.matmul(out=sdps[:, jj * H:(jj + 1) * H],
                                             lhsT=ST[:, t, :], rhs=sblk[:],
                                             start=True, stop=True)
                        # w = exp(0.5*shs*shd); msg *= w  (in place, per tile)
                        for jj, t in enumerate(tiles):
                            nc.vector.tensor_tensor(
                                out=G[:, t, 0:H], in0=G[:, t, 0:H],
                                in1=sdps[:, jj * H:(jj + 1) * H],
                                op=mybir.AluOpType.mult)
                        for jj, t in enumerate(tiles):
                            nc.scalar.activation(
                                out=G[:, t, 0:H], in_=G[:, t, 0:H],
                                func=mybir.ActivationFunctionType.Exp,
                                scale=0.5)
                            nc.vector.tensor_tensor(
                                out=G[:, t, H:RW].rearrange(
                                    "p (h c) -> p h c", h=H),
                                in0=G[:, t, H:RW].rearrange(
                                    "p (h c) -> p h c", h=H),
                                in1=G[:, t, 0:H, None].to_broadcast(
                                    [128, H, C]),
                                op=mybir.AluOpType.mult)
                        acc = ps_acc.tile([128, RW1], F32, space="PSUM",
                                          tag="acc")
                        nc.tensor.matmul(out=acc[:, 0:RW], lhsT=identb[:],
                                         rhs=rhss[:, 0:RW], start=True,
                                         stop=(nb == 0))
                        for jj, t in enumerate(tiles):
                            nc.tensor.matmul(
                                out=acc[:, 0:RW], lhsT=S[:, t, :],
                                rhs=G[:, t, 0:RW], start=False,
                                stop=(jj == nb - 1))
                        if layer == 1:
                            epilogue1(b, acc)
                        else:
                            epilogue2(b, acc)
                    if layer == 1:
                        for k in range(4):
                            if b0c < agb[k + 1] <= b1c:
                                ag_slice(T2S, T2F, k)

            def epilogue1(b, acc):
                nb0 = b * 128
                P = min(128, SH - nb0)
                rz = sm.tile([128, H], F32, tag="rz")
                nc.vector.reciprocal(out=rz[:], in_=acc[:, 0:H])
                o1 = wk.tile([128, D1], F32, tag="o1")
                nc.vector.tensor_tensor(
                    out=o1[:].rearrange("p (h c) -> p h c", h=H),
                    in0=acc[:, H:H + D1].rearrange("p (h c) -> p h c", h=H),
                    in1=rz[:, :, None].to_broadcast([128, H, C1]),
                    op=mybir.AluOpType.mult)
                nc.vector.tensor_tensor(out=o1[:], in0=o1[:], in1=b1b[:],
                                        op=mybir.AluOpType.add)
                r1 = wk.tile([128, D1], F32, tag="r1")
                nc.scalar.activation(out=r1[:], in_=o1[:],
                                     func=mybir.ActivationFunctionType.Relu,
                                     scale=-1.0)
                ew = wk.tile([128, D1], F32, tag="ew")
                nc.scalar.activation(out=ew[:], in_=r1[:],
                                     func=mybir.ActivationFunctionType.Exp,
                                     scale=-1.0)
                rp = wk.tile([128, D1], F32, tag="rp")
                nc.scalar.activation(out=rp[:], in_=o1[:],
                                     func=mybir.ActivationFunctionType.Relu)
                hact = wk.tile([128, D1], BF16, tag="hact")
                nc.vector.scalar_tensor_tensor(
                    out=hact[:], in0=ew[:], scalar=-1.0, in1=rp[:],
                    op0=mybir.AluOpType.add, op1=mybir.AluOpType.add)
                tp = ps_tp.tile([128, 128], BF16, space="PSUM", tag="tp")
                nc.tensor.transpose(out=tp[:], in_=hact[:], identity=identb[:])
                hT = wk.tile([128, 128], BF16, tag="hTT")
                nc.vector.tensor_copy(out=hT[:], in_=tp[:])
                h2ps = ps_mm.tile([128, D2], F32, space="PSUM", tag="mm")
                nc.tensor.matmul(out=h2ps[:], lhsT=hT[:], rhs=W2sb[:],
                                 start=True, stop=True)
                t2 = sm.tile([128, D2], F32, tag="t2")
                nc.vector.tensor_tensor(out=t2[:], in0=h2ps[:], in1=att2b[:],
                                        op=mybir.AluOpType.mult)
                s2 = sm.tile([128, H], F32, tag="s2")
                nc.vector.tensor_reduce(
                    out=s2[:], in_=t2[:].rearrange("p (h c) -> p h c", h=H),
                    axis=mybir.AxisListType.X, op=mybir.AluOpType.add)
                row2 = wk.tile([128, E2ROW], BF16, tag="row2")
                nc.vector.memset(row2[:, RW2:], 0.0)
                nc.scalar.activation(out=row2[:, 0:H], in_=s2[:],
                                     func=mybir.ActivationFunctionType.Tanh,
                                     scale=0.5)
                nc.vector.tensor_copy(out=row2[:, H:RW2], in_=h2ps[:])
                nc.sync.dma_start(T2S[nb0:nb0 + P, :], row2[:P, :])

            def epilogue2(b, acc):
                nb0 = b * 128
                P = min(128, SH - nb0)
                rz = sm.tile([128, H], F32, tag="rz")
                nc.vector.reciprocal(out=rz[:], in_=acc[:, 0:H])
                o2 = sm.tile([128, D2], F32, tag="o2")
                nc.vector.tensor_tensor(
                    out=o2[:].rearrange("p (h c) -> p h c", h=H),
                    in0=acc[:, H:H + D2].rearrange("p (h c) -> p h c", h=H),
                    in1=rz[:, :, None].to_broadcast([128, H, C2]),
                    op=mybir.AluOpType.mult)
                red = sm.tile([128, C2], F32, tag="red")
                nc.vector.tensor_reduce(
                    out=red[:], in_=o2[:].rearrange("p (h c) -> p c h", h=H),
                    axis=mybir.AxisListType.X, op=mybir.AluOpType.add)
                fin = sm.tile([128, C2], F32, tag="fin")
                nc.vector.scalar_tensor_tensor(
                    out=fin[:], in0=red[:], scalar=1.0 / H, in1=b2b[:],
                    op0=mybir.AluOpType.mult, op1=mybir.AluOpType.add)
                nc.sync.dma_start(out[nb0:nb0 + P, :], fin[:P, :])

            # ---- layer 1 (AG1 slices already issued inside phase 1)
            edge_layer(T1F, T1S, E1ROW, D1, C1, RW1, 1)
            # ---- layer 2 (AG2 slices issued inside layer-1 chunk loop)
            edge_layer(T2F, T2S, E2ROW, D2, C2, RW2, 2)

    nc.compile()
    return nc


_CACHE = {}


def kernel(x, edge_index, W1, att1, b1, W2, att2, b2, cfg: Cfg | None = None,
           trace: bool = False):
    cfg = cfg or Cfg()
    in_maps, layout = host_prep(cfg, x, edge_index, W1, att1, b1, W2, att2, b2)
    key = (cfg.N, cfg.IN, cfg.H, cfg.C1, cfg.C2, layout[0], layout[1])
    if key not in _CACHE:
        _CACHE[key] = build(cfg, layout)
    nc = _CACHE[key]
    r = run_bass_kernel_spmd(nc, in_maps, core_ids=list(range(cfg.NC)),
                             trace=trace)
    outp = np.concatenate([r.results[c]["out"] for c in range(cfg.NC)], axis=0)
    if trace:
        kernel.last_exec_time_ns = r.exec_time_ns
    return outp.astype(np.float32)


# revision 8
# speedup vs baseline: 1.0516x; 1.0014x over previous
"""OODGAT 2-layer GNN kernel for 8 Trainium2 NeuronCores — v5.

Deltas vs v3 baseline (which was gather-descriptor-bound, GpSimd 75% busy):
  - Per-tile indirect DMAs (1664 x ~1.5us serial) replaced by dma_gather
    instructions covering 8 tiles (1024 rows) each; edge slots per block are
    split into low/high-src tile runs so indices fit int16 (<32768).
  - Table rows padded to 256B/512B multiples for dma_gather (descriptor
    drain rate is size-insensitive, so padding is free).
  - One-hot S and S^T matrices are precomputed on host and streamed
    sequentially from DRAM (bandwidth rides under the descriptor-paced
    gathers), removing the per-tile PE transpose + PSUM copy and the
    per-chunk is_equal builds.
  - Slot layout, S/S^T, and idx grids are shared by both layers.
"""
import numpy as np
import ml_dtypes
from dataclasses import dataclass

import concourse.bass as bass
import concourse.bacc as bacc
import concourse.mybir as mybir
import concourse.tile as tile
from concourse.bass_utils import run_bass_kernel_spmd

F32 = mybir.dt.float32
BF16 = mybir.dt.bfloat16
I16 = mybir.dt.int16
BF = ml_dtypes.bfloat16

E1ROW = 256   # T1 row: [sh1(4) | h1(128) | 0pad] bf16 -> 512B
E2ROW = 128   # T2 row: [sh2(4) | h2(32) | 0pad] bf16 -> 256B
GMAX = 7      # tiles per dma_gather (896 rows; margin below 1024-desc ring)
AGB = (0, 13, 26, 39)   # AllGather slice boundaries (blocks)


@dataclass
class Cfg:
    N: int = 50000
    IN: int = 256
    H: int = 4
    C1: int = 32
    C2: int = 8
    NC: int = 8
    CH_MAX: int = 48   # max tiles per chunk
    SPLIT: int = 32768

    @property
    def D1(self):
        return self.H * self.C1

    @property
    def D2(self):
        return self.H * self.C2

    @property
    def SH(self):
        return self.N // self.NC

    @property
    def NBLK(self):
        return (self.SH + 127) // 128


def _wrap_idx16(flat):
    n = len(flat)
    core = flat.reshape(n // 16, 16).T.astype(np.int16)
    return np.tile(core, (8, 1))


def host_prep(cfg: Cfg, x, edge_index, W1, att1, b1, W2, att2, b2):
    N, SH, NBLK, NC, SPLIT = cfg.N, cfg.SH, cfg.NBLK, cfg.NC, cfg.SPLIT
    src = np.asarray(edge_index[0], dtype=np.int64)
    dst = np.asarray(edge_index[1], dtype=np.int64)
    # self-loops handled by the per-block identity matmul (no gather)

    # slice-major permuted row ids so chunked AllGathers write contiguously
    agb = list(AGB) + [NBLK]
    r0s = np.array([a * 128 for a in agb[:-1]])
    r1s = np.array([min(a * 128, SH) for a in agb[1:]])
    srk = r1s - r0s
    offs = np.concatenate([[0], np.cumsum(NC * srk)[:-1]])
    slice_of = np.zeros(SH, np.int64)
    for k in range(len(srk)):
        slice_of[r0s[k]:r1s[k]] = k
    rloc = src % SH
    kk = slice_of[rloc]
    prow = offs[kk] + (src // SH) * srk[kk] + (rloc - r0s[kk])

    core_of = dst // SH
    per_core = []
    kbL = np.zeros(NBLK, np.int64)
    kbH = np.zeros(NBLK, np.int64)
    for c in range(NC):
        m = core_of == c
        s_c, d_c = prow[m], dst[m] - c * SH
        lo = s_c < SPLIT
        per_core.append((s_c, d_c, lo))
        for sel, kb in ((lo, kbL), (~lo, kbH)):
            blkcnt = np.bincount(d_c[sel] // 128, minlength=NBLK)
            np.maximum(kb, blkcnt, out=kb)
    kbL = -(-kbL // 128)
    kbH = -(-kbH // 128)

    # chunks of whole blocks; per chunk tiles ordered [L tiles | H tiles]
    chunks = []   # (b0, b1, t0, ntL, ntH)
    b0 = 0
    t0 = 0
    while b0 < NBLK:
        bend = b0 + 1
        while bend < NBLK and (kbL[b0:bend + 1].sum() + kbH[b0:bend + 1].sum()
                               ) <= cfg.CH_MAX:
            bend += 1
        ntL = int(kbL[b0:bend].sum())
        ntH = int(kbH[b0:bend].sum())
        chunks.append((b0, bend, t0, ntL, ntH))
        t0 += ntL + ntH
        b0 = bend
    T = t0

    ident_b = np.eye(128, dtype=np.float32).astype(BF)
    att1_b = np.broadcast_to(np.asarray(att1, np.float32).reshape(
        1, cfg.D1), (128, cfg.D1)).copy()
    att2_b = np.broadcast_to(np.asarray(att2, np.float32).reshape(
        1, cfg.D2), (128, cfg.D2)).copy()
    b1_b = np.broadcast_to(np.asarray(b1, np.float32).reshape(
        1, cfg.D1), (128, cfg.D1)).copy()
    b2_b = np.broadcast_to(np.asarray(b2, np.float32).reshape(
        1, cfg.C2), (128, cfg.C2)).copy()
    W1_b = np.asarray(W1, np.float32).astype(BF)
    W2_b = np.asarray(W2, np.float32).astype(BF)
    x = np.asarray(x, np.float32)

    in_maps = []
    for c in range(NC):
        s_c, d_c, lo_c = per_core[c]
        idxf = np.zeros(T * 128, np.int64)
        Smat = np.zeros((128, T, 128), np.float32)
        for (b0c, b1c, tc0, ntL, ntH) in chunks:
            tL = tc0
            tH = tc0 + ntL
            for b in range(b0c, b1c):
                for lo, kb, tref in ((True, kbL, 'L'), (False, kbH, 'H')):
                    m = (d_c // 128 == b) & (lo_c == lo)
                    sv = s_c[m] - (0 if lo else SPLIT)
                    ld = d_c[m] - b * 128
                    t0b = tL if lo else tH
                    n = len(sv)
                    lanes = np.arange(n) % 128
                    cols = t0b + np.arange(n) // 128
                    idxf[cols * 128 + lanes] = sv
                    Smat[lanes, cols, ld] = 1.0
                    if lo:
                        tL += int(kb[b])
                    else:
                        tH += int(kb[b])
        idx16 = _wrap_idx16(idxf)
        Sb = Smat.astype(BF)
        STb = np.ascontiguousarray(Smat.transpose(2, 1, 0)).astype(BF)
        xT = np.ascontiguousarray(x[c * SH:(c + 1) * SH].T).astype(BF)
        in_maps.append(dict(
            xT=xT, idx16=idx16,
            Sm=Sb.reshape(128, T * 128),
            STm=STb.reshape(128, T * 128),
            ident=ident_b, W1=W1_b, W2=W2_b,
            att1_b=att1_b, att2_b=att2_b, b1_b=b1_b, b2_b=b2_b,
        ))
    layout = (tuple(int(v) for v in kbL), tuple(int(v) for v in kbH),
              tuple(chunks))
    return in_maps, layout


def build(cfg: Cfg, layout):
    kbL, kbH, chunks = layout
    N, SH, NBLK, NC = cfg.N, cfg.SH, cfg.NBLK, cfg.NC
    IN, H, C1, C2, D1, D2 = cfg.IN, cfg.H, cfg.C1, cfg.C2, cfg.D1, cfg.D2
    SPLIT = cfg.SPLIT
    RW1 = H + D1            # 132
    RW2 = H + D2            # 36
    T = sum(kbL) + sum(kbH)
    CHMAX = max(c[3] + c[4] for c in chunks)
    KIN = IN // 128

    nc = bacc.Bacc("TRN2", target_bir_lowering=False, debug=False,
                   enable_asserts=True, num_devices=NC)

    xT_in = nc.dram_tensor("xT", [IN, SH], BF16, kind="ExternalInput")
    idx_in = nc.dram_tensor("idx16", [128, T * 8], I16, kind="ExternalInput")
    S_in = nc.dram_tensor("Sm", [128, T * 128], BF16, kind="ExternalInput")
    ST_in = nc.dram_tensor("STm", [128, T * 128], BF16, kind="ExternalInput")
    ident_in = nc.dram_tensor("ident", [128, 128], BF16, kind="ExternalInput")
    W1_in = nc.dram_tensor("W1", [IN, D1], BF16, kind="ExternalInput")
    W2_in = nc.dram_tensor("W2", [D1, D2], BF16, kind="ExternalInput")
    att1_in = nc.dram_tensor("att1_b", [128, D1], F32, kind="ExternalInput")
    att2_in = nc.dram_tensor("att2_b", [128, D2], F32, kind="ExternalInput")
    b1_in = nc.dram_tensor("b1_b", [128, D1], F32, kind="ExternalInput")
    b2_in = nc.dram_tensor("b2_b", [128, C2], F32, kind="ExternalInput")
    out = nc.dram_tensor("out", [SH, C2], F32, kind="ExternalOutput")

    T1S = nc.dram_tensor("T1S", [SH, E1ROW], BF16, kind="Internal")
    T1F = nc.dram_tensor("T1F", [N, E1ROW], BF16, kind="Internal",
                         addr_space="Shared")
    T2S = nc.dram_tensor("T2S", [SH, E2ROW], BF16, kind="Internal")
    T2F = nc.dram_tensor("T2F", [N, E2ROW], BF16, kind="Internal",
                         addr_space="Shared")

    with tile.TileContext(nc) as tc:
        with tc.tile_pool(name="res", bufs=1) as res, \
             tc.tile_pool(name="gp", bufs=2) as gp, \
             tc.tile_pool(name="sp", bufs=2) as sp, \
             tc.tile_pool(name="stp", bufs=2) as stp, \
             tc.tile_pool(name="wk", bufs=3) as wk, \
             tc.tile_pool(name="sm", bufs=4) as sm, \
             tc.tile_pool(name="ps_acc", bufs=2, space="PSUM") as ps_acc, \
             tc.tile_pool(name="ps_sd", bufs=2, space="PSUM") as ps_sd, \
             tc.tile_pool(name="ps_mm", bufs=2, space="PSUM") as ps_mm, \
             tc.tile_pool(name="ps_tp", bufs=2, space="PSUM") as ps_tp:

            # ---- resident constants
            idx16 = res.tile([128, T * 8], I16)
            identb = res.tile([128, 128], BF16)
            att1b = res.tile([128, D1], F32)
            att2b = res.tile([128, D2], F32)
            b1b = res.tile([128, D1], F32)
            b2b = res.tile([128, C2], F32)
            W2sb = res.tile([D1, D2], BF16)
            nc.sync.dma_start(idx16[:], idx_in[:])
            nc.sync.dma_start(identb[:], ident_in[:])
            nc.sync.dma_start(att1b[:], att1_in[:])
            nc.sync.dma_start(att2b[:], att2_in[:])
            nc.sync.dma_start(b1b[:], b1_in[:])
            nc.sync.dma_start(b2b[:], b2_in[:])
            nc.sync.dma_start(W2sb[:], W2_in[:])
            xTs, W1s = [], []
            for k in range(KIN):
                t_ = res.tile([128, SH], BF16, tag=f"xT{k}")
                nc.sync.dma_start(t_[:], xT_in[k * 128:(k + 1) * 128, :])
                xTs.append(t_)
                w_ = res.tile([128, D1], BF16, tag=f"W1{k}")
                nc.sync.dma_start(w_[:], W1_in[k * 128:(k + 1) * 128, :])
                W1s.append(w_)

            # AllGather slice boundaries (blocks) for overlap
            agb = list(AGB) + [NBLK]
            ag_r0 = [a * 128 for a in agb[:-1]]
            ag_r1 = [min(a * 128, SH) for a in agb[1:]]
            ag_off = [0]
            for k in range(3):
                ag_off.append(ag_off[-1] + NC * (ag_r1[k] - ag_r0[k]))

            def ag_slice(TS, TF, k):
                r0, r1 = ag_r0[k], ag_r1[k]
                nc.gpsimd.collective_compute(
                    "AllGather", mybir.AluOpType.bypass,
                    replica_groups=[list(range(NC))],
                    ins=[TS[r0:r1, :]],
                    outs=[TF[ag_off[k]:ag_off[k] + NC * (r1 - r0), :]])

            # ---- phase 1: rows [tanh(.5*h1.att1) | h1 | 0] in bf16
            for i in range(NBLK):
                n0 = i * 128
                P = min(128, SH - n0)
                h1ps = ps_mm.tile([128, D1], F32, space="PSUM", tag="mm")
                for k in range(KIN):
                    nc.tensor.matmul(out=h1ps[:P, :], lhsT=xTs[k][:, n0:n0 + P],
                                     rhs=W1s[k][:], start=(k == 0),
                                     stop=(k == KIN - 1))
                tmp = wk.tile([128, D1], F32, tag="tmp")
                nc.vector.tensor_tensor(out=tmp[:P, :], in0=h1ps[:P, :],
                                        in1=att1b[:P, :],
                                        op=mybir.AluOpType.mult)
                s1 = sm.tile([128, H], F32, tag="s1")
                nc.vector.tensor_reduce(
                    out=s1[:P, :],
                    in_=tmp[:P, :].rearrange("p (h c) -> p h c", h=H),
                    axis=mybir.AxisListType.X, op=mybir.AluOpType.add)
                row = wk.tile([128, E1ROW], BF16, tag="row")
                nc.vector.memset(row[:, RW1:], 0.0)
                nc.scalar.activation(out=row[:P, 0:H], in_=s1[:P, :],
                                     func=mybir.ActivationFunctionType.Tanh,
                                     scale=0.5)
                nc.vector.tensor_copy(out=row[:P, H:RW1], in_=h1ps[:P, :])
                nc.sync.dma_start(T1S[n0:n0 + P, :], row[:P, :])
                for k in range(4):
                    if i == agb[k + 1] - 1:
                        ag_slice(T1S, T1F, k)

            def edge_layer(full_tbl, shard_tbl, EROW, D, C, RW, layer):
                TL = full_tbl[0:SPLIT, :]
                TH = full_tbl[SPLIT:N, :]
                for (b0c, b1c, tc0, ntL, ntH) in chunks:
                    nt = ntL + ntH
                    G = gp.tile([128, CHMAX, EROW], BF16, tag="G")
                    for (TB, j0, j1) in ((TL, 0, ntL), (TH, ntL, nt)):
                        j = j0
                        while j < j1:
                            je = min(j + GMAX, j1)
                            nidx = (je - j) * 128
                            nc.gpsimd.dma_gather(
                                G[:, j:je, :], TB[:],
                                idx16[:, (tc0 + j) * 8:(tc0 + je) * 8],
                                nidx, nidx, EROW)
                            j = je
                    S = sp.tile([128, CHMAX, 128], BF16, tag="S")
                    nc.sync.dma_start(
                        S[:, 0:nt, :],
                        S_in[:, tc0 * 128:(tc0 + nt) * 128].rearrange(
                            "p (t d) -> p t d", d=128))
                    ST = stp.tile([128, CHMAX, 128], BF16, tag="ST")
                    nc.sync.dma_start(
                        ST[:, 0:nt, :],
                        ST_in[:, tc0 * 128:(tc0 + nt) * 128].rearrange(
                            "p (t d) -> p t d", d=128))

                    tL = 0
                    tH = ntL
                    for b in range(b0c, b1c):
                        nbL, nbH = kbL[b], kbH[b]
                        tiles = list(range(tL, tL + nbL)) + \
                            list(range(tH, tH + nbH))
                        tL += nbL
                        tH += nbH
                        nb = len(tiles)
                        nb0 = b * 128
                        P = min(128, SH - nb0)
                        sblk = sm.tile([128, H], BF16, tag="sblk")
                        nc.vector.memset(sblk[:], 0.0)
                        nc.scalar.dma_start(sblk[:P, :],
                                            shard_tbl[nb0:nb0 + P, 0:H])
                        Gs = wk.tile([128, RW1], BF16, tag="Gs")
                        nc.vector.memset(Gs[:], 0.0)
                        nc.scalar.dma_start(Gs[:P, 0:RW],
                                            shard_tbl[nb0:nb0 + P, 0:RW])
                        # self-loop: w = exp(0.5*sh^2), rhss = [w | msg*w]
                        ps_self = sm.tile([128, H], BF16, tag="ps_self")
                        nc.vector.tensor_tensor(out=ps_self[:], in0=sblk[:],
                                                in1=sblk[:],
                                                op=mybir.AluOpType.mult)
                        rhss = wk.tile([128, RW1], BF16, tag="rhss")
                        nc.scalar.activation(
                            out=rhss[:, 0:H], in_=ps_self[:],
                            func=mybir.ActivationFunctionType.Exp, scale=0.5)
                        nc.vector.tensor_tensor(
                            out=rhss[:, H:RW].rearrange(
                                "p (h c) -> p h c", h=H),
                            in0=Gs[:, H:RW].rearrange("p (h c) -> p h c", h=H),
                            in1=rhss[:, 0:H, None].to_broadcast([128, H, C]),
                            op=mybir.AluOpType.mult)
                        # dst-side sh expansion via streamed S^T
                        sdps = ps_sd.tile([128, 64 * H], F32, space="PSUM",
                                          tag="sd")
                        for jj, t in enumerate(tiles):
                            nc.tensor.matmul(out=sdps[:, jj * H:(jj + 1) * H],
                                             lhsT=ST[:, t, :], rhs=sblk[:],
                                             start=True, stop=True)
                        # w = exp(0.5*shs*shd); msg *= w  (in place, per tile)
                        for jj, t in enumerate(tiles):
                            nc.vector.tensor_tensor(
                                out=G[:, t, 0:H], in0=G[:, t, 0:H],
                                in1=sdps[:, jj * H:(jj + 1) * H],
                                op=mybir.AluOpType.mult)
                        for jj, t in enumerate(tiles):
                            nc.scalar.activation(
                                out=G[:, t, 0:H], in_=G[:, t, 0:H],
                                func=mybir.ActivationFunctionType.Exp,
                                scale=0.5)
                            nc.vector.tensor_tensor(
                                out=G[:, t, H:RW].rearrange(
                                    "p (h c) -> p h c", h=H),
                                in0=G[:, t, H:RW].rearrange(
                                    "p (h c) -> p h c", h=H),
                                in1=G[:, t, 0:H, None].to_broadcast(
                                    [128, H, C]),
                                op=mybir.AluOpType.mult)
                        acc = ps_acc.tile([128, RW1], F32, space="PSUM",
                                          tag="acc")
                        nc.tensor.matmul(out=acc[:, 0:RW], lhsT=identb[:],
                                         rhs=rhss[:, 0:RW], start=True,
                                         stop=(nb == 0))
                        for jj, t in enumerate(tiles):
                            nc.tensor.matmul(
                                out=acc[:, 0:RW], lhsT=S[:, t, :],
                                rhs=G[:, t, 0:RW], start=False,
                                stop=(jj == nb - 1))
                        if layer == 1:
                            epilogue1(b, acc)
                        else:
                            epilogue2(b, acc)
                    if layer == 1:
                        for k in range(4):
                            if b0c < agb[k + 1] <= b1c:
                                ag_slice(T2S, T2F, k)

            def epilogue1(b, acc):
                nb0 = b * 128
                P = min(128, SH - nb0)
                rz = sm.tile([128, H], F32, tag="rz")
                nc.vector.reciprocal(out=rz[:], in_=acc[:, 0:H])
                o1 = wk.tile([128, D1], F32, tag="o1")
                nc.vector.tensor_tensor(
                    out=o1[:].rearrange("p (h c) -> p h c", h=H),
                    in0=acc[:, H:H + D1].rearrange("p (h c) -> p h c", h=H),
                    in1=rz[:, :, None].to_broadcast([128, H, C1]),
                    op=mybir.AluOpType.mult)
                nc.vector.tensor_tensor(out=o1[:], in0=o1[:], in1=b1b[:],
                                        op=mybir.AluOpType.add)
                r1 = wk.tile([128, D1], F32, tag="r1")
                nc.scalar.activation(out=r1[:], in_=o1[:],
                                     func=mybir.ActivationFunctionType.Relu,
                                     scale=-1.0)
                ew = wk.tile([128, D1], F32, tag="ew")
                nc.scalar.activation(out=ew[:], in_=r1[:],
                                     func=mybir.ActivationFunctionType.Exp,
                                     scale=-1.0)
                rp = wk.tile([128, D1], F32, tag="rp")
                nc.scalar.activation(out=rp[:], in_=o1[:],
                                     func=mybir.ActivationFunctionType.Relu)
                hact = wk.tile([128, D1], BF16, tag="hact")
                nc.vector.scalar_tensor_tensor(
                    out=hact[:], in0=ew[:], scalar=-1.0, in1=rp[:],
                    op0=mybir.AluOpType.add, op1=mybir.AluOpType.add)
                tp = ps_tp.tile([128, 128], BF16, space="PSUM", tag="tp")
                nc.tensor.transpose(out=tp[:], in_=hact[:], identity=identb[:])
                hT = wk.tile([128, 128], BF16, tag="hTT")
                nc.vector.tensor_copy(out=hT[:], in_=tp[:])
                h2ps = ps_mm.tile([128, D2], F32, space="PSUM", tag="mm")
                nc.tensor.matmul(out=h2ps[:], lhsT=hT[:], rhs=W2sb[:],
                                 start=True, stop=True)
                t2 = sm.tile([128, D2], F32, tag="t2")
                nc.vector.tensor_tensor(out=t2[:], in0=h2ps[:], in1=att2b[:],
                                        op=mybir.AluOpType.mult)
                s2 = sm.tile([128, H], F32, tag="s2")
                nc.vector.tensor_reduce(
                    out=s2[:], in_=t2[:].rearrange("p (h c) -> p h c", h=H),
                    axis=mybir.AxisListType.X, op=mybir.AluOpType.add)
                row2 = wk.tile([128, E2ROW], BF16, tag="row2")
                nc.vector.memset(row2[:, RW2:], 0.0)
                nc.scalar.activation(out=row2[:, 0:H], in_=s2[:],
                                     func=mybir.ActivationFunctionType.Tanh,
                                     scale=0.5)
                nc.vector.tensor_copy(out=row2[:, H:RW2], in_=h2ps[:])
                nc.sync.dma_start(T2S[nb0:nb0 + P, :], row2[:P, :])

            def epilogue2(b, acc):
                nb0 = b * 128
                P = min(128, SH - nb0)
                rz = sm.tile([128, H], F32, tag="rz")
                nc.vector.reciprocal(out=rz[:], in_=acc[:, 0:H])
                o2 = sm.tile([128, D2], F32, tag="o2")
                nc.vector.tensor_tensor(
                    out=o2[:].rearrange("p (h c) -> p h c", h=H),
                    in0=acc[:, H:H + D2].rearrange("p (h c) -> p h c", h=H),
                    in1=rz[:, :, None].to_broadcast([128, H, C2]),
                    op=mybir.AluOpType.mult)
                red = sm.tile([128, C2], F32, tag="red")
                nc.vector.tensor_reduce(
                    out=red[:], in_=o2[:].rearrange("p (h c) -> p c h", h=H),
                    axis=mybir.AxisListType.X, op=mybir.AluOpType.add)
                fin = sm.tile([128, C2], F32, tag="fin")
                nc.vector.scalar_tensor_tensor(
                    out=fin[:], in0=red[:], scalar=1.0 / H, in1=b2b[:],
                    op0=mybir.AluOpType.mult, op1=mybir.AluOpType.add)
                nc.sync.dma_start(out[nb0:nb0 + P, :], fin[:P, :])

            # ---- layer 1 (AG1 slices already issued inside phase 1)
            edge_layer(T1F, T1S, E1ROW, D1, C1, RW1, 1)
            # ---- layer 2 (AG2 slices issued inside layer-1 chunk loop)
            edge_layer(T2F, T2S, E2ROW, D2, C2, RW2, 2)

    nc.compile()
    return nc


_CACHE = {}


def kernel(x, edge_index, W1, att1, b1, W2, att2, b2, cfg: Cfg | None = None,
           trace: bool = False):
    cfg = cfg or Cfg()
    in_maps, layout = host_prep(cfg, x, edge_index, W1, att1, b1, W2, att2, b2)
    key = (cfg.N, cfg.IN, cfg.H, cfg.C1, cfg.C2, layout[0], layout[1])
    if key not in _CACHE:
        _CACHE[key] = build(cfg, layout)
    nc = _CACHE[key]
    r = run_bass_kernel_spmd(nc, in_maps, core_ids=list(range(cfg.NC)),
                             trace=trace)
    outp = np.concatenate([r.results[c]["out"] for c in range(cfg.NC)], axis=0)
    if trace:
        kernel.last_exec_time_ns = r.exec_time_ns
    return outp.astype(np.float32)
